# revision 8
# baseline (speedup 1.0000x reference)
"""AutoCorrelation layer kernel for 8 Trainium2 NeuronCores.

Math note: the reference's rfft/irfft pair over the zero-padded head dim
computes a circular cross-correlation; its mean over all lags collapses
analytically to (sum_d q_proj) * (sum_d k_proj) per head.  So
corr_mean[b,l] = (1/(H*L)) * sum_h (q[b,l] @ WqS + bqS)_h * (k[b,l] @ WkS + bkS)_h
with WqS = Wq.reshape(D,H,DK).sum(-1).  Everything downstream (top-6,
softmax, gather, output projection) follows the reference directly.

Distribution: all 8 cores redundantly compute the cheap preprocessing
(full q/k/v, ~24MB) and each core computes its own column shard of the
huge (256, 262144) output projection (column-parallel, no collectives).
"""
import sys
import types

sys.path.insert(0, "/opt/trn_rl_repo")

import numpy as np
import concourse.bass as bass
import concourse.mybir as mybir
import concourse.tile as tile
from concourse import bacc
from concourse.bass_utils import run_bass_kernel_spmd
from concourse.masks import make_identity

F32 = mybir.dt.float32
BF16 = mybir.dt.bfloat16

N_CORES = 8
B, L, D, H, DK = 8, 1024, 256, 8, 32
K_TOP = 6
NSH = (L * D) // N_CORES          # 32768 output cols per core
TILE_N = 2048
N_TILES = NSH // TILE_N           # 16
SUBS = TILE_N // 512              # 4
SCALE = 1.0 / (H * L)

WP_BUFS = 5
DEBUG_OUTS = False
TRACE = False          # test harness sets this for profiled runs
LAST_RESULT = None     # stashed BassKernelResults from the last kernel() call

_CACHE = {}


def _build_nc():
    nc = bacc.Bacc("TRN2", target_bir_lowering=False, debug=False, num_devices=N_CORES)

    q_d = nc.dram_tensor("q", [B * L, D], F32, kind="ExternalInput").ap()
    k_d = nc.dram_tensor("k", [B * L, D], F32, kind="ExternalInput").ap()
    v_d = nc.dram_tensor("v", [B * L, D], F32, kind="ExternalInput").ap()
    wq_d = nc.dram_tensor("wq", [D, D], F32, kind="ExternalInput").ap()
    wk_d = nc.dram_tensor("wk", [D, D], F32, kind="ExternalInput").ap()
    wv_d = nc.dram_tensor("wv", [D, D], F32, kind="ExternalInput").ap()
    bq_d = nc.dram_tensor("bq", [1, D], F32, kind="ExternalInput").ap()
    bk_d = nc.dram_tensor("bk", [1, D], F32, kind="ExternalInput").ap()
    bv_d = nc.dram_tensor("bv", [1, D], F32, kind="ExternalInput").ap()
    wp_d = nc.dram_tensor("wp", [D, NSH], F32, kind="ExternalInput").ap()
    bp_d = nc.dram_tensor("bp", [1, NSH], F32, kind="ExternalInput").ap()
    out_d = nc.dram_tensor("out", [B, NSH], F32, kind="ExternalOutput").ap()
    if DEBUG_OUTS:
        dbg_r = nc.dram_tensor("dbg_r", [B, L], F32, kind="ExternalOutput").ap()
        dbg_aggt = nc.dram_tensor("dbg_aggt", [128, 16], F32, kind="ExternalOutput").ap()

    with tile.TileContext(nc) as tc:
        with (
            tc.tile_pool(name="cst", bufs=1) as cst,
            tc.tile_pool(name="work", bufs=2) as work,
            tc.tile_pool(name="wpp", bufs=WP_BUFS) as wpp,
            tc.tile_pool(name="outp", bufs=3) as outp,
            tc.tile_pool(name="ps_tp", bufs=2, space="PSUM") as ps_tp,
            tc.tile_pool(name="ps_big", bufs=1, space="PSUM") as ps_big,
            tc.tile_pool(name="ps_out", bufs=2, space="PSUM") as ps_out,
        ):
            # ---------------- constants / weights ----------------
            ident128 = cst.tile([128, 128], F32)
            make_identity(nc, ident128[:, :])
            ident8 = cst.tile([8, 8], F32)
            make_identity(nc, ident8[:, :])
            one1 = cst.tile([1, 1], F32)
            nc.vector.memset(one1[:, :], 1.0)
            ones8f = cst.tile([1, 8], F32)
            nc.vector.memset(ones8f[:, :], 1.0)
            ones8b = cst.tile([1, 8], BF16)
            nc.vector.memset(ones8b[:, :], 1.0)
            # blk3[h, b, m] = SCALE * (m == b): per-batch column selector for the
            # corr reduction over heads (host-provided structural constant)
            blk3_d = nc.dram_tensor("blk3", [8, 8, 8], F32, kind="ExternalInput").ap()
            blk3 = cst.tile([8, 8, 8], F32)
            nc.sync.dma_start(blk3[:, :, :], blk3_d)

            wq_sb = cst.tile([128, 2, 256], F32)
            nc.sync.dma_start(wq_sb[:, :, :], wq_d.rearrange("(c p) d -> p c d", p=128))
            wk_sb = cst.tile([128, 2, 256], F32)
            nc.sync.dma_start(wk_sb[:, :, :], wk_d.rearrange("(c p) d -> p c d", p=128))
            wv_sb = cst.tile([128, 2, 256], F32)
            nc.sync.dma_start(wv_sb[:, :, :], wv_d.rearrange("(c p) d -> p c d", p=128))
            bq_sb = cst.tile([1, 256], F32)
            nc.sync.dma_start(bq_sb[:, :], bq_d)
            bk_sb = cst.tile([1, 256], F32)
            nc.sync.dma_start(bk_sb[:, :], bk_d)
            bv_sb = cst.tile([1, 256], F32)
            nc.sync.dma_start(bv_sb[:, :], bv_d)

            # head-sums of projection weights: WqS[d, h] = sum_z Wq[d, h*32+z]
            wqs = cst.tile([128, 2, 8], F32)
            nc.vector.reduce_sum(out=wqs[:, :, :],
                                 in_=wq_sb[:, :, :].rearrange("p c (h z) -> p c h z", z=DK),
                                 axis=mybir.AxisListType.X)
            wks = cst.tile([128, 2, 8], F32)
            nc.vector.reduce_sum(out=wks[:, :, :],
                                 in_=wk_sb[:, :, :].rearrange("p c (h z) -> p c h z", z=DK),
                                 axis=mybir.AxisListType.X)
            bqs_row = cst.tile([1, 8], F32)
            nc.vector.reduce_sum(out=bqs_row[:, :],
                                 in_=bq_sb[:, :].rearrange("o (h z) -> o h z", z=DK),
                                 axis=mybir.AxisListType.X)
            bks_row = cst.tile([1, 8], F32)
            nc.vector.reduce_sum(out=bks_row[:, :],
                                 in_=bk_sb[:, :].rearrange("o (h z) -> o h z", z=DK),
                                 axis=mybir.AxisListType.X)
            # [1,8] -> [8,1] via K=1 matmul against [1,1] ones
            bqs_ps = ps_tp.tile([8, 1], F32, tag="tp")
            nc.tensor.matmul(bqs_ps[:, :], bqs_row[:, :], one1[:, :], start=True, stop=True)
            bqs_vert = cst.tile([8, 1], F32)
            nc.vector.tensor_copy(bqs_vert[:, :], bqs_ps[:, :])
            bks_ps = ps_tp.tile([8, 1], F32, tag="tp")
            nc.tensor.matmul(bks_ps[:, :], bks_row[:, :], one1[:, :], start=True, stop=True)
            bks_vert = cst.tile([8, 1], F32)
            nc.vector.tensor_copy(bks_vert[:, :], bks_ps[:, :])

            # v (bf16, cast in DMA) for the weighted gather
            v_all = cst.tile([128, B, 8, 256], BF16)
            nc.gpsimd.dma_start(v_all[:, :, :, :],
                                v_d.rearrange("(b t p) d -> p b t d", p=128, t=8))

            # ---------------- per-batch q/k projections ----------------
            ps_r = ps_big.tile([8, L], F32, tag="big")
            for b in range(B):
                xsT = {}
                for (nat_src, w_sum, bias_v, nm) in (
                    (q_d, wqs, bqs_vert, "q"),
                    (k_d, wks, bks_vert, "k"),
                ):
                    nat = work.tile([128, 8, 256], F32, tag="nat")
                    nc.sync.dma_start(
                        nat[:, :, :],
                        nat_src[L * b:L * (b + 1), :].rearrange("(t p) d -> p t d", p=128))
                    # transpose to [d, l] chunks via PE
                    tr = work.tile([128, 2, L], F32, tag="tr")
                    for t in range(8):
                        for c in range(2):
                            tp = ps_tp.tile([128, 128], F32, tag="tp")
                            nc.tensor.transpose(tp[:, :], nat[:, t, 128 * c:128 * (c + 1)], ident128[:, :])
                            nc.vector.tensor_copy(tr[:, c, 128 * t:128 * (t + 1)], tp[:, :])
                    # project: xsT[h, l] = sum_d WS[d, h] * xT[d, l]
                    xs = work.tile([8, L], F32, tag=f"{nm}sT")
                    for half in range(2):
                        sl = slice(512 * half, 512 * (half + 1))
                        ps_x = ps_big.tile([8, 512], F32, tag="proj", bufs=2)
                        nc.tensor.matmul(ps_x[:, :], w_sum[:, 0, :], tr[:, 0, sl], start=True, stop=False)
                        nc.tensor.matmul(ps_x[:, :], w_sum[:, 1, :], tr[:, 1, sl], start=False, stop=True)
                        # psum->sbuf with per-head bias add
                        nc.vector.tensor_scalar(
                            out=xs[:, sl], in0=ps_x[:, :],
                            scalar1=bias_v[:, 0:1], scalar2=None, op0=mybir.AluOpType.add)
                    xsT[nm] = xs
                # prod_b[h, l] then accumulate into corr rows via blk3 selector
                prod = work.tile([8, L], F32, tag="prod")
                nc.vector.tensor_mul(prod[:, :], xsT["q"][:, :], xsT["k"][:, :])
                for half in range(2):
                    sl = slice(512 * half, 512 * (half + 1))
                    nc.tensor.matmul(ps_r[:, sl], blk3[:, b, :], prod[:, sl],
                                     start=(b == 0), stop=(b == B - 1))

            # ---------------- corr, top-6, softmax, select ----------------
            r_sb = cst.tile([8, L], F32)
            nc.vector.tensor_copy(r_sb[:, :], ps_r[:, :])
            if DEBUG_OUTS:
                nc.sync.dma_start(dbg_r, r_sb[:, :])

            topv = cst.tile([8, 8], F32)
            nc.vector.max(topv[:, :], r_sb[:, :])
            negm0 = cst.tile([8, 1], F32)
            nc.vector.tensor_scalar_mul(negm0[:, :], topv[:, 0:1], -1.0)
            e_sb = cst.tile([8, K_TOP], F32)
            nc.scalar.activation(e_sb[:, :], topv[:, 0:K_TOP],
                                 mybir.ActivationFunctionType.Exp,
                                 bias=negm0[:, 0:1], scale=1.0)
            z_sb = cst.tile([8, 1], F32)
            nc.vector.reduce_sum(out=z_sb[:, :], in_=e_sb[:, :], axis=mybir.AxisListType.X)
            zinv = cst.tile([8, 1], F32)
            nc.vector.reciprocal(zinv[:, :], z_sb[:, :])
            w_sb = cst.tile([8, K_TOP], F32)
            nc.vector.tensor_scalar_mul(w_sb[:, :], e_sb[:, :], zinv[:, 0:1])

            # selu[b, l] = sum_j w_j * (r[b, l] == topv[b, j])
            selu = cst.tile([8, L], F32)
            ohw = cst.tile([8, L], F32)
            for j in range(K_TOP):
                dst = selu if j == 0 else ohw
                nc.vector.tensor_scalar(
                    out=dst[:, :], in0=r_sb[:, :],
                    scalar1=topv[:, j:j + 1], scalar2=w_sb[:, j:j + 1],
                    op0=mybir.AluOpType.is_equal, op1=mybir.AluOpType.mult)
                if j > 0:
                    nc.vector.tensor_add(selu[:, :], selu[:, :], ohw[:, :])

            # transpose sel to [l_local, t] layout (bf16), t = l // 128
            selT = cst.tile([128, 64], BF16)
            for t in range(8):
                tp8 = ps_tp.tile([128, 8], F32, tag="tp")
                nc.tensor.transpose(tp8[:, :], selu[0:8, 128 * t:128 * (t + 1)], ident8[:, :])
                nc.vector.tensor_copy(selT[:, 8 * t:8 * (t + 1)], tp8[:, :])

            # vbarT[e, b] = sum_l v[b, l, e] * sel[b, l]
            vbarT = cst.tile([128, 16], F32)
            for b in range(B):
                for e in range(2):
                    ps_v = ps_tp.tile([128, 1], F32, tag="tp")
                    for t in range(8):
                        nc.tensor.matmul(ps_v[:, :],
                                         v_all[:, b, t, 128 * e:128 * (e + 1)],
                                         selT[:, 8 * t + b:8 * t + b + 1],
                                         start=(t == 0), stop=(t == 7))
                    nc.vector.tensor_copy(vbarT[:, 8 * e + b:8 * e + b + 1], ps_v[:, :])

            # aggT[d', b] = sum_e Wv[e, d'] * vbarT[e, b] + bv[d']   (bf16 out)
            aggt_bf = cst.tile([128, 16], BF16)
            for m in range(2):
                ps_a = ps_tp.tile([128, 8], F32, tag="tp")
                nc.tensor.matmul(ps_a[:, :], wv_sb[:, 0, 128 * m:128 * (m + 1)],
                                 vbarT[:, 0:8], start=True, stop=False)
                nc.tensor.matmul(ps_a[:, :], wv_sb[:, 1, 128 * m:128 * (m + 1)],
                                 vbarT[:, 8:16], start=False, stop=False)
                nc.tensor.matmul(ps_a[:, :], bv_sb[0:1, 128 * m:128 * (m + 1)],
                                 ones8f[:, :], start=False, stop=True)
                nc.vector.tensor_copy(aggt_bf[:, 8 * m:8 * (m + 1)], ps_a[:, :])
            if DEBUG_OUTS:
                aggt_f = cst.tile([128, 16], F32)
                nc.vector.tensor_copy(aggt_f[:, :], aggt_bf[:, :])
                nc.sync.dma_start(dbg_aggt, aggt_f[:, :])

            # ---------------- big output projection (column shard) ----------------
            for nt in range(N_TILES):
                ncol = slice(TILE_N * nt, TILE_N * (nt + 1))
                wp0 = wpp.tile([128, TILE_N], BF16, tag="wp0")
                nc.gpsimd.dma_start(wp0[:, :], wp_d[0:128, ncol])
                wp1 = wpp.tile([128, TILE_N], BF16, tag="wp1")
                nc.gpsimd.dma_start(wp1[:, :], wp_d[128:256, ncol])
                bp_t = wpp.tile([1, TILE_N], BF16, tag="bp", bufs=2)
                nc.gpsimd.dma_start(bp_t[:, :], bp_d[0:1, ncol])
                o_sb = outp.tile([8, TILE_N], F32)
                for s in range(SUBS):
                    ssl = slice(512 * s, 512 * (s + 1))
                    ps = ps_out.tile([8, 512], F32, tag="po")
                    nc.tensor.matmul(ps[:, :], aggt_bf[:, 0:8], wp0[:, ssl], start=True, stop=False)
                    nc.tensor.matmul(ps[:, :], aggt_bf[:, 8:16], wp1[:, ssl], start=False, stop=False)
                    nc.tensor.matmul(ps[:, :], ones8b[:, :], bp_t[0:1, ssl], start=False, stop=True)
                    nc.scalar.copy(o_sb[:, ssl], ps[:, :])
                nc.sync.dma_start(out_d[:, ncol], o_sb[:, :])

    nc.finalize()
    return nc


def _get_nc():
    if "nc" not in _CACHE:
        _CACHE["nc"] = _build_nc()
    return _CACHE["nc"]


def kernel(queries, keys, values, Wq, bq, Wk, bk, Wv, bv, Wp, bp):
    queries = np.ascontiguousarray(np.asarray(queries, np.float32).reshape(B * L, D))
    keys = np.ascontiguousarray(np.asarray(keys, np.float32).reshape(B * L, D))
    values = np.ascontiguousarray(np.asarray(values, np.float32).reshape(B * L, D))
    Wq = np.ascontiguousarray(np.asarray(Wq, np.float32))
    Wk = np.ascontiguousarray(np.asarray(Wk, np.float32))
    Wv = np.ascontiguousarray(np.asarray(Wv, np.float32))
    bq = np.asarray(bq, np.float32).reshape(1, D)
    bk = np.asarray(bk, np.float32).reshape(1, D)
    bv = np.asarray(bv, np.float32).reshape(1, D)
    Wp = np.asarray(Wp, np.float32)
    bp = np.asarray(bp, np.float32)

    nc = _get_nc()
    blk3_const = np.zeros((8, 8, 8), np.float32)
    for b in range(B):
        blk3_const[:, b, b] = SCALE
    in_maps = []
    for i in range(N_CORES):
        cols = slice(NSH * i, NSH * (i + 1))
        in_maps.append({
            "q": queries, "k": keys, "v": values, "blk3": blk3_const,
            "wq": Wq, "wk": Wk, "wv": Wv,
            "bq": bq, "bk": bk, "bv": bv,
            "wp": np.ascontiguousarray(Wp[:, cols]),
            "bp": np.ascontiguousarray(bp[cols]).reshape(1, NSH),
        })
    res = run_bass_kernel_spmd(nc, in_maps, core_ids=list(range(N_CORES)), trace=TRACE)
    global LAST_RESULT
    LAST_RESULT = res
    out = np.concatenate([res.results[i]["out"] for i in range(N_CORES)], axis=1)
    return out.reshape(B, L, D)
